# revision 26
# baseline (speedup 1.0000x reference)
"""BEV encoder on 8 Trainium2 NeuronCores.

Pipeline: 5M points -> 4x250x250 BEV grid (scatter max/min/count/intensity)
-> 3x conv3x3+BN+ReLU (4->32->64->64).

Device algorithm (everything in ONE NEFF, data-parallel over points):
  * Points are tiled 128-per-matmul. For each tile, one-hot row/col matrices
    over the 250 y-bins / x-bins are built with a single DVE is_equal each;
    `onehot_y^T @ (onehot_x * w)` then scatter-accumulates all channels of a
    tile into PSUM grids (bf16 matmul, fp32 accumulate).
  * scatter-max/min are not matmul-expressible, so they are computed by a
    2-pass log-sum-exp refinement: pass A accumulates sum(e^{+-16 z}) per bin
    (-> bound m1, err <= log(n)/16), pass B gathers m1 per point (PE
    transpose-broadcast + one-hot matmul + fused multiply-reduce) and
    accumulates sum(e^{+-300 (z - m1)}) -> max/min with err ~ log(n)/300.
    Validated vs the exact reference: final rel err ~1e-3 (tol 2e-2).
  * Grids are AllReduced (sum - LSE makes max/min sum-reducible too) so every
    core holds the global BEV; the small CNN runs replicated in fp32 (exact
    BN batch stats; conv bias skipped - it cancels in BN), with each core
    DMA-ing out only its 32-row slice of the output.
"""
import sys, os, time
sys.path.insert(0, "/opt/trn_rl_repo")
import numpy as np

N_CORES = 8
BEV = 250
T_TILES = int(os.environ.get("BEV_TILES", "4896"))   # tiles of 128 points per core
U = 32                                               # tiles per hw-loop iteration
PTS_PER_CORE = 128 * T_TILES
N_PAD = N_CORES * PTS_PER_CORE
K1 = 16.0
K2 = 300.0
TINY = 1e-30
EPS = 1e-5
LW = 252                                             # padded image row width
LFLAT = 252 * 252                                    # 63504
IMG_PAD = 63520
N_CHUNK = 125                                        # 2-row conv chunks
U_CNN = 25

_CACHE = {}
LAST_HW_EXEC_NS = None


def _build():
    import concourse.bass as bass
    import concourse.tile as tile
    from concourse import mybir
    from concourse.bass import ds

    f32 = mybir.dt.float32
    i32 = mybir.dt.int32
    bf = mybir.dt.bfloat16
    Alu = mybir.AluOpType
    Act = mybir.ActivationFunctionType
    T = T_TILES

    import concourse.bacc as bacc
    nc = bacc.Bacc("TRN2", num_devices=N_CORES)
    pts_t = nc.dram_tensor("pts", [128, T * 4], f32, kind="ExternalInput")
    iota_t = nc.dram_tensor("iota250", [128, 250], bf, kind="ExternalInput")
    ident_t = nc.dram_tensor("ident", [128, 128], f32, kind="ExternalInput")
    i125_t = nc.dram_tensor("i125", [125, 2], f32, kind="ExternalInput")
    w1_t = nc.dram_tensor("w1", [4, 9 * 32], f32, kind="ExternalInput")
    w2_t = nc.dram_tensor("w2", [32, 9 * 64], f32, kind="ExternalInput")
    w3_t = nc.dram_tensor("w3", [64, 9 * 64], f32, kind="ExternalInput")
    gb1_t = nc.dram_tensor("gb1", [32, 2], f32, kind="ExternalInput")
    gb2_t = nc.dram_tensor("gb2", [64, 2], f32, kind="ExternalInput")
    gb3_t = nc.dram_tensor("gb3", [64, 2], f32, kind="ExternalInput")
    roff_t = nc.dram_tensor("roff", [1, 1], mybir.dt.uint32, kind="ExternalInput")
    out_t = nc.dram_tensor("out", [64, 32, 250], f32, kind="ExternalOutput")
    dbg_t = nc.dram_tensor("dbg", [125, 4000], f32, kind="ExternalOutput")

    groups = [list(range(N_CORES))]

    with tile.TileContext(nc) as tc:
        with tc.tile_pool(name="dram", bufs=1, space="DRAM") as drp, \
             tc.tile_pool(name="const", bufs=1) as cp:
            iota = cp.tile([128, 250], bf, tag="iota")
            ident = cp.tile([128, 128], f32, tag="ident")
            i125 = cp.tile([125, 2], f32, tag="i125")
            nc.sync.dma_start(out=iota[:], in_=iota_t[:])
            nc.sync.dma_start(out=ident[:], in_=ident_t[:])
            nc.sync.dma_start(out=i125[:], in_=i125_t[:])
            tinyc = cp.tile([128, 1], f32, tag="tinyc")
            epsc = cp.tile([128, 1], f32, tag="epsc")
            nc.vector.memset(tinyc[:], TINY)
            nc.vector.memset(epsc[:], EPS)
            zeroL = cp.tile([128, 125], bf, tag="zeroL")
            zeroR = cp.tile([128, 500], bf, tag="zeroR")
            nc.vector.memset(zeroL[:], 0.0)
            nc.vector.memset(zeroR[:], 0.0)
            GA = cp.tile([125, 2000], f32, tag="GA")
            G0 = cp.tile([125, 500], f32, tag="G0")
            G1 = cp.tile([125, 500], f32, tag="G1")
            GB = cp.tile([125, 1000], f32, tag="GB")
            bev_dram = drp.tile([4, 250, 250], f32, tag="bev")
            y3_dram = drp.tile([64, 250, 252], f32, tag="y3")

            with tc.tile_pool(name="ptsarr", bufs=1) as ap_:
                cols = ap_.tile([128, T, 7], f32, tag="cols")

                # ---------------- prep: unpack + per-point quantities --------
                NB = 32
                B = T // NB
                with tc.tile_pool(name="prep", bufs=2) as pp:
                    nc.vector.memset(cols[:], 0.0)
                    for b in range(NB if os.environ.get("BEV_PREP", "1") == "1" else 0):
                        sl = slice(b * B, (b + 1) * B)
                        raw = pp.tile([128, B, 4], f32, tag="raw")
                        nc.gpsimd.dma_start(out=raw[:], in_=pts_t[:, b * B * 4:(b + 1) * B * 4])
                        x_ap, y_ap = raw[:, :, 0], raw[:, :, 1]
                        z_ap, it_ap = raw[:, :, 2], raw[:, :, 3]
                        tmp = pp.tile([128, B], f32, tag="tmp")
                        frac = pp.tile([128, B], f32, tag="frac")
                        # xi = clip(trunc((x+50)/0.4), 0, 249)
                        ti = pp.tile([128, B], i32, tag="ti")
                        fx = pp.tile([128, B], f32, tag="fx")
                        nc.vector.tensor_scalar(out=tmp[:], in0=x_ap, scalar1=50.0,
                                                scalar2=2.5, op0=Alu.add, op1=Alu.mult)
                        nc.vector.tensor_copy(out=ti[:], in_=tmp[:])
                        nc.vector.tensor_copy(out=frac[:], in_=ti[:])
                        nc.vector.tensor_tensor(out=fx[:], in0=frac[:], in1=tmp[:], op=Alu.is_gt)
                        nc.vector.tensor_tensor(out=tmp[:], in0=frac[:], in1=fx[:], op=Alu.subtract)
                        nc.vector.tensor_scalar(out=cols[:, sl, 0], in0=tmp[:], scalar1=0.0,
                                                scalar2=249.0, op0=Alu.max, op1=Alu.min)
                        # valid mask
                        vx = pp.tile([128, B], f32, tag="vx")
                        v2 = pp.tile([128, B], f32, tag="v2")
                        nc.vector.tensor_scalar(out=vx[:], in0=x_ap, scalar1=-50.0,
                                                scalar2=None, op0=Alu.is_ge)
                        nc.vector.tensor_scalar(out=v2[:], in0=x_ap, scalar1=50.0,
                                                scalar2=None, op0=Alu.is_lt)
                        nc.vector.tensor_tensor(out=vx[:], in0=vx[:], in1=v2[:], op=Alu.mult)
                        nc.vector.tensor_scalar(out=v2[:], in0=y_ap, scalar1=-50.0,
                                                scalar2=None, op0=Alu.is_ge)
                        nc.vector.tensor_tensor(out=vx[:], in0=vx[:], in1=v2[:], op=Alu.mult)
                        nc.vector.tensor_scalar(out=v2[:], in0=y_ap, scalar1=50.0,
                                                scalar2=None, op0=Alu.is_lt)
                        nc.vector.tensor_tensor(out=vx[:], in0=vx[:], in1=v2[:], op=Alu.mult)
                        # yi = (clip(trunc((y+50)/0.4),0,249) + 1)*v - 1
                        nc.vector.tensor_scalar(out=tmp[:], in0=y_ap, scalar1=50.0,
                                                scalar2=2.5, op0=Alu.add, op1=Alu.mult)
                        nc.vector.tensor_copy(out=ti[:], in_=tmp[:])
                        nc.vector.tensor_copy(out=frac[:], in_=ti[:])
                        nc.vector.tensor_tensor(out=fx[:], in0=frac[:], in1=tmp[:], op=Alu.is_gt)
                        nc.vector.tensor_tensor(out=tmp[:], in0=frac[:], in1=fx[:], op=Alu.subtract)
                        nc.vector.tensor_scalar(out=tmp[:], in0=tmp[:], scalar1=0.0,
                                                scalar2=249.0, op0=Alu.max, op1=Alu.min)
                        nc.vector.tensor_scalar(out=tmp[:], in0=tmp[:], scalar1=1.0,
                                                scalar2=None, op0=Alu.add)
                        nc.vector.tensor_tensor(out=tmp[:], in0=tmp[:], in1=vx[:], op=Alu.mult)
                        nc.vector.tensor_scalar(out=cols[:, sl, 1], in0=tmp[:], scalar1=1.0,
                                                scalar2=None, op0=Alu.subtract)
                        # zp = (z+4)*v - 4 ; zm = (z-6)*v + 6
                        zp = pp.tile([128, B], f32, tag="zp")
                        zm = pp.tile([128, B], f32, tag="zm")
                        nc.vector.tensor_scalar(out=zp[:], in0=z_ap, scalar1=4.0,
                                                scalar2=None, op0=Alu.add)
                        nc.vector.tensor_tensor(out=zp[:], in0=zp[:], in1=vx[:], op=Alu.mult)
                        nc.vector.tensor_scalar(out=zp[:], in0=zp[:], scalar1=4.0,
                                                scalar2=None, op0=Alu.subtract)
                        nc.vector.tensor_scalar(out=zm[:], in0=z_ap, scalar1=6.0,
                                                scalar2=None, op0=Alu.subtract)
                        nc.vector.tensor_tensor(out=zm[:], in0=zm[:], in1=vx[:], op=Alu.mult)
                        nc.vector.tensor_scalar(out=zm[:], in0=zm[:], scalar1=6.0,
                                                scalar2=None, op0=Alu.add)
                        nc.vector.tensor_scalar(out=cols[:, sl, 5], in0=zp[:], scalar1=K2,
                                                scalar2=None, op0=Alu.mult)
                        nc.vector.tensor_scalar(out=cols[:, sl, 6], in0=zm[:], scalar1=-K2,
                                                scalar2=None, op0=Alu.mult)
                        nc.scalar.activation(out=cols[:, sl, 3], in_=zp[:], func=Act.Exp, scale=K1)
                        nc.scalar.activation(out=cols[:, sl, 4], in_=zp[:], func=Act.Exp, scale=-K1)
                        nc.vector.tensor_copy(out=cols[:, sl, 2], in_=it_ap)

                # ---------------- sweep A: counts / isum / e^{+-16z} ---------
                with tc.tile_pool(name="psA", bufs=1, space="PSUM") as psA, \
                     tc.tile_pool(name="swa", bufs=4) as sa:
                    pA = [psA.tile([125, 500], f32, tag=f"pa{i}", name=f"pa{i}")
                          for i in range(4)]
                    for p in pA:
                        nc.tensor.matmul(out=p[:], lhsT=zeroL[:], rhs=zeroR[:],
                                         start=True, stop=False)
                    _swa_on = os.environ.get("BEV_SWA", "1") == "1"
                    with tc.For_i(0, T, U) as i0:
                        stA = sa.tile([128, U, 7], f32, tag="stgA")
                        nc.vector.tensor_copy(out=stA[:], in_=cols[:, ds(i0, U), :])
                        for u in range(U if _swa_on else 0):
                            rhs1 = sa.tile([128, 500], bf, tag="r1")
                            rhs2 = sa.tile([128, 500], bf, tag="r2")
                            ey = sa.tile([128, 250], bf, tag="ey")
                            nc.vector.tensor_scalar(out=rhs1[:, 0:250], in0=iota[:],
                                                    scalar1=stA[:, u, 0:1], scalar2=None,
                                                    op0=Alu.is_equal)
                            nc.vector.tensor_scalar(out=rhs1[:, 250:500], in0=rhs1[:, 0:250],
                                                    scalar1=stA[:, u, 2:3], scalar2=None,
                                                    op0=Alu.mult)
                            nc.vector.tensor_scalar(out=rhs2[:, 0:250], in0=rhs1[:, 0:250],
                                                    scalar1=stA[:, u, 3:4], scalar2=None,
                                                    op0=Alu.mult)
                            nc.vector.tensor_scalar(out=rhs2[:, 250:500], in0=rhs1[:, 0:250],
                                                    scalar1=stA[:, u, 4:5], scalar2=None,
                                                    op0=Alu.mult)
                            nc.vector.tensor_scalar(out=ey[:], in0=iota[:],
                                                    scalar1=stA[:, u, 1:2], scalar2=None,
                                                    op0=Alu.is_equal)
                            nc.tensor.matmul(out=pA[0][:], lhsT=ey[:, 0:125], rhs=rhs1[:],
                                             start=False, stop=False)
                            nc.tensor.matmul(out=pA[2][:], lhsT=ey[:, 0:125], rhs=rhs2[:],
                                             start=False, stop=False)
                            nc.tensor.matmul(out=pA[1][:], lhsT=ey[:, 125:250], rhs=rhs1[:],
                                             start=False, stop=False)
                            nc.tensor.matmul(out=pA[3][:], lhsT=ey[:, 125:250], rhs=rhs2[:],
                                             start=False, stop=False)
                    for i, p in enumerate(pA):
                        nc.tensor.matmul(out=p[:], lhsT=zeroL[:], rhs=zeroR[:],
                                         start=False, stop=True)
                        nc.vector.tensor_copy(out=GA[:, 500 * i:500 * (i + 1)], in_=p[:])

                # AllReduce sweep-A grids
                ccin = drp.tile([125, 2000], f32, tag="ccinA")
                ccout = drp.tile([125, 2000], f32, tag="ccoutA")
                nc.gpsimd.dma_start(out=ccin[:], in_=GA[:])
                if os.environ.get("BEV_CC", "1") == "1":
                    nc.gpsimd.collective_compute("AllReduce", Alu.add, replica_groups=groups,
                                                 ins=[ccin[:]], outs=[ccout[:]])
                else:
                    nc.sync.dma_start(out=ccout[:], in_=ccin[:])
                nc.sync.dma_start(out=GA[:], in_=ccout[:])

                # m1 grids: G0/G1 = [m1p | m1m] per y-chunk.
                # exact ln via exponent extraction: ACT Ln LUT only covers
                # ~[2^-65, 2^65]; E1 spans e^{+-80}.
                LN2 = 0.6931471805599453
                with tc.tile_pool(name="lnp", bufs=2) as lnpool:
                    def ln_scaled(dst, src_ap, s):
                        eb = lnpool.tile([125, 250], i32, tag="eb")
                        ef = lnpool.tile([125, 250], f32, tag="ef")
                        mb = lnpool.tile([125, 250], i32, tag="mb")
                        lnm = lnpool.tile([125, 250], f32, tag="lnm")
                        nc.vector.tensor_scalar(out=eb[:], in0=src_ap.bitcast(i32),
                                                scalar1=23, scalar2=None,
                                                op0=Alu.logical_shift_right)
                        nc.vector.tensor_copy(out=ef[:], in_=eb[:])
                        nc.vector.tensor_scalar(out=mb[:], in0=src_ap.bitcast(i32),
                                                scalar1=0x7FFFFF, scalar2=0x3F800000,
                                                op0=Alu.bitwise_and, op1=Alu.bitwise_or)
                        nc.scalar.activation(out=lnm[:], in_=mb[:].bitcast(f32), func=Act.Ln)
                        nc.vector.tensor_scalar(out=ef[:], in0=ef[:], scalar1=127.0,
                                                scalar2=LN2 * s, op0=Alu.subtract,
                                                op1=Alu.mult)
                        nc.vector.tensor_scalar(out=lnm[:], in0=lnm[:], scalar1=s,
                                                scalar2=None, op0=Alu.mult)
                        nc.vector.tensor_tensor(out=dst, in0=ef[:], in1=lnm[:], op=Alu.add)

                    for c, Gc in ((0, G0), (1, G1)):
                        ln_scaled(Gc[:, 0:250], GA[:, 1000 + 500 * c:1250 + 500 * c], 1.0 / K1)
                        ln_scaled(Gc[:, 250:500], GA[:, 1250 + 500 * c:1500 + 500 * c], -1.0 / K1)

                # ---------------- sweep B: gather m1, scatter e^{K2(z-m1)} --
                with tc.tile_pool(name="psB", bufs=1, space="PSUM") as psB, \
                     tc.tile_pool(name="psW", bufs=2, space="PSUM") as psW, \
                     tc.tile_pool(name="swb", bufs=4) as sb:
                    pB = [psB.tile([125, 500], f32, tag=f"pb{i}", name=f"pb{i}")
                          for i in range(2)]
                    for p in pB:
                        nc.tensor.matmul(out=p[:], lhsT=zeroL[:], rhs=zeroR[:],
                                         start=True, stop=False)
                    _swb_stage = int(os.environ.get("BEV_SWB_N", "5"))
                    _swb_on = os.environ.get("BEV_SWB", "1") == "1"
                    with tc.For_i(0, T, U) as i0:
                        stB = sb.tile([128, U, 7], f32, tag="stgB")
                        nc.vector.tensor_copy(out=stB[:], in_=cols[:, ds(i0, U), :])
                        for u in range(U if _swb_on else 0):
                            pT = psW.tile([128, 128], f32, tag="pT")
                            nc.tensor.transpose(out=pT[:],
                                                in_=stB[:, u, 1:2].to_broadcast([128, 128]),
                                                identity=ident[:])
                            eyT0 = sb.tile([125, 128], f32, tag="eyT0")
                            eyT1 = sb.tile([125, 128], f32, tag="eyT1")
                            nc.vector.tensor_scalar(out=eyT0[:], in0=pT[0:125, :],
                                                    scalar1=i125[:, 0:1], scalar2=None,
                                                    op0=Alu.is_equal)
                            nc.vector.tensor_scalar(out=eyT1[:], in0=pT[0:125, :],
                                                    scalar1=i125[:, 1:2], scalar2=None,
                                                    op0=Alu.is_equal)
                            if _swb_stage < 2:
                                continue
                            pW = psW.tile([128, 500], f32, tag="pW")
                            nc.tensor.matmul(out=pW[:], lhsT=eyT0[:], rhs=G0[:],
                                             start=True, stop=False)
                            nc.tensor.matmul(out=pW[:], lhsT=eyT1[:], rhs=G1[:],
                                             start=False, stop=True)
                            if _swb_stage < 3:
                                scrx = sb.tile([128, 500], bf, tag="scrx")
                                nc.vector.tensor_copy(out=scrx[:], in_=pW[:])
                                continue
                            ex = sb.tile([128, 250], bf, tag="ex")
                            nc.vector.tensor_scalar(out=ex[:], in0=iota[:],
                                                    scalar1=stB[:, u, 0:1], scalar2=None,
                                                    op0=Alu.is_equal)
                            wsb = sb.tile([128, 500], f32, tag="wsb")
                            nc.vector.tensor_copy(out=wsb[:], in_=pW[:])
                            mp = sb.tile([128, 1], f32, tag="mp")
                            mm = sb.tile([128, 1], f32, tag="mm")
                            scr = sb.tile([128, 250], bf, tag="scr")
                            scr2 = sb.tile([128, 250], bf, tag="scr2")
                            nc.vector.scalar_tensor_tensor(out=scr[:], in0=wsb[:, 0:250],
                                                           scalar=1.0, in1=ex[:],
                                                           op0=Alu.mult, op1=Alu.mult,
                                                           accum_out=mp[:])
                            nc.vector.scalar_tensor_tensor(out=scr2[:], in0=wsb[:, 250:500],
                                                           scalar=1.0, in1=ex[:],
                                                           op0=Alu.mult, op1=Alu.mult,
                                                           accum_out=mm[:])
                            if _swb_stage < 4:
                                continue
                            e2p = sb.tile([128, 1], f32, tag="e2p")
                            e2m = sb.tile([128, 1], f32, tag="e2m")
                            nc.scalar.activation(out=e2p[:], in_=mp[:], func=Act.Exp,
                                                 scale=-K2, bias=stB[:, u, 5:6])
                            nc.scalar.activation(out=e2m[:], in_=mm[:], func=Act.Exp,
                                                 scale=K2, bias=stB[:, u, 6:7])
                            rhsB = sb.tile([128, 500], bf, tag="rB")
                            nc.vector.tensor_scalar(out=rhsB[:, 0:250], in0=ex[:],
                                                    scalar1=e2p[:], scalar2=None, op0=Alu.mult)
                            nc.vector.tensor_scalar(out=rhsB[:, 250:500], in0=ex[:],
                                                    scalar1=e2m[:], scalar2=None, op0=Alu.mult)
                            if _swb_stage < 5:
                                continue
                            eyb = sb.tile([128, 250], bf, tag="eyb")
                            nc.vector.tensor_scalar(out=eyb[:], in0=iota[:],
                                                    scalar1=stB[:, u, 1:2], scalar2=None,
                                                    op0=Alu.is_equal)
                            nc.tensor.matmul(out=pB[0][:], lhsT=eyb[:, 0:125], rhs=rhsB[:],
                                             start=False, stop=False)
                            nc.tensor.matmul(out=pB[1][:], lhsT=eyb[:, 125:250], rhs=rhsB[:],
                                             start=False, stop=False)
                    for i, p in enumerate(pB):
                        nc.tensor.matmul(out=p[:], lhsT=zeroL[:], rhs=zeroR[:],
                                         start=False, stop=True)
                        nc.vector.tensor_copy(out=GB[:, 500 * i:500 * (i + 1)], in_=p[:])

                ccinB = drp.tile([125, 1000], f32, tag="ccinB")
                ccoutB = drp.tile([125, 1000], f32, tag="ccoutB")
                nc.gpsimd.dma_start(out=ccinB[:], in_=GB[:])
                if os.environ.get("BEV_CC", "1") == "1":
                    nc.gpsimd.collective_compute("AllReduce", Alu.add, replica_groups=groups,
                                                 ins=[ccinB[:]], outs=[ccoutB[:]])
                else:
                    nc.sync.dma_start(out=ccoutB[:], in_=ccinB[:])
                nc.sync.dma_start(out=GB[:], in_=ccoutB[:])

                # one-off gather debug for tile 0
                with tc.tile_pool(name="dbg1", bufs=1) as dgp, \
                     tc.tile_pool(name="dbgp", bufs=1, space="PSUM") as dpp:
                    dpT = dpp.tile([128, 128], f32, tag="dpT")
                    nc.tensor.transpose(out=dpT[:],
                                        in_=cols[:, 0, 1:2].to_broadcast([128, 128]),
                                        identity=ident[:])
                    deyT0 = dgp.tile([125, 128], f32, tag="deyT0")
                    deyT1 = dgp.tile([125, 128], f32, tag="deyT1")
                    nc.vector.tensor_scalar(out=deyT0[:], in0=dpT[0:125, :],
                                            scalar1=i125[:, 0:1], scalar2=None,
                                            op0=Alu.is_equal)
                    nc.vector.tensor_scalar(out=deyT1[:], in0=dpT[0:125, :],
                                            scalar1=i125[:, 1:2], scalar2=None,
                                            op0=Alu.is_equal)
                    dpW = dpp.tile([128, 500], f32, tag="dpW")
                    nc.tensor.matmul(out=dpW[:], lhsT=deyT0[:], rhs=G0[:],
                                     start=True, stop=False)
                    nc.tensor.matmul(out=dpW[:], lhsT=deyT1[:], rhs=G1[:],
                                     start=False, stop=True)
                    dwsb = dgp.tile([128, 500], f32, tag="dwsb")
                    nc.vector.tensor_copy(out=dwsb[:], in_=dpW[:])
                    dex = dgp.tile([128, 250], bf, tag="dex")
                    nc.vector.tensor_scalar(out=dex[:], in0=iota[:],
                                            scalar1=cols[:, 0, 0:1], scalar2=None,
                                            op0=Alu.is_equal)
                    dmp = dgp.tile([128, 1], f32, tag="dmp")
                    dscr = dgp.tile([128, 250], bf, tag="dscr")
                    nc.vector.scalar_tensor_tensor(out=dscr[:], in0=dwsb[:, 0:250],
                                                   scalar=1.0, in1=dex[:],
                                                   op0=Alu.mult, op1=Alu.mult,
                                                   accum_out=dmp[:])
                    de2p = dgp.tile([128, 1], f32, tag="de2p")
                    nc.scalar.activation(out=de2p[:], in_=dmp[:], func=Act.Exp,
                                         scale=-K2, bias=cols[:, 0, 5:6])
                    dpack = dgp.tile([128, 8], f32, tag="dpack")
                    nc.vector.tensor_copy(out=dpack[:, 0:1], in_=dmp[:])
                    nc.vector.tensor_copy(out=dpack[:, 1:2], in_=de2p[:])
                    nc.vector.tensor_copy(out=dpack[:, 2:8], in_=cols[:, 0, 0:6])
                    nc.sync.dma_start(out=dbg_t[:, 2000:2500], in_=dwsb[0:125, :])
                    nc.sync.dma_start(out=dbg_t[:, 2500:2508], in_=dpack[0:125, :])
                nc.sync.dma_start(out=dbg_t[:, 0:2000], in_=GA[:])
                nc.sync.dma_start(out=dbg_t[:, 3000:3500], in_=G0[:])
                nc.sync.dma_start(out=dbg_t[:, 3500:4000], in_=G1[:])
                # ---------------- BEV assembly --------------------------------
                with tc.tile_pool(name="bev", bufs=1) as bvp:
                    for c in (0, 1):
                        C_c = GA[:, 500 * c:500 * c + 250]
                        I_c = GA[:, 500 * c + 250:500 * c + 500]
                        Gc = G0 if c == 0 else G1
                        E2p_c = GB[:, 500 * c:500 * c + 250]
                        E2m_c = GB[:, 500 * c + 250:500 * c + 500]
                        mask = bvp.tile([125, 250], f32, tag=f"mask{c}")
                        ch = bvp.tile([125, 250], f32, tag=f"ch{c}")
                        nc.vector.tensor_scalar(out=mask[:], in0=C_c, scalar1=0.5,
                                                scalar2=None, op0=Alu.is_ge)
                        # ch0 = mask * (m1p + ln(E2p+tiny)/K2)
                        nc.scalar.activation(out=ch[:], in_=E2p_c, func=Act.Ln, bias=tinyc[0:125, :])
                        nc.vector.tensor_scalar(out=ch[:], in0=ch[:], scalar1=1.0 / K2,
                                                scalar2=None, op0=Alu.mult)
                        nc.vector.tensor_tensor(out=ch[:], in0=ch[:], in1=Gc[:, 0:250], op=Alu.add)
                        nc.vector.tensor_tensor(out=ch[:], in0=ch[:], in1=mask[:], op=Alu.mult)
                        nc.sync.dma_start(out=bev_dram[0, 125 * c:125 * (c + 1), :], in_=ch[:])
                        ch1 = bvp.tile([125, 250], f32, tag=f"ch1_{c}")
                        nc.scalar.activation(out=ch1[:], in_=E2m_c, func=Act.Ln, bias=tinyc[0:125, :])
                        nc.vector.tensor_scalar(out=ch1[:], in0=ch1[:], scalar1=-1.0 / K2,
                                                scalar2=None, op0=Alu.mult)
                        nc.vector.tensor_tensor(out=ch1[:], in0=ch1[:], in1=Gc[:, 250:500], op=Alu.add)
                        nc.vector.tensor_tensor(out=ch1[:], in0=ch1[:], in1=mask[:], op=Alu.mult)
                        nc.sync.dma_start(out=bev_dram[1, 125 * c:125 * (c + 1), :], in_=ch1[:])
                        ch2 = bvp.tile([125, 250], f32, tag=f"ch2_{c}")
                        nc.scalar.activation(out=ch2[:], in_=C_c, func=Act.Ln, bias=1.0)
                        nc.sync.dma_start(out=bev_dram[2, 125 * c:125 * (c + 1), :], in_=ch2[:])
                        ch3 = bvp.tile([125, 250], f32, tag=f"ch3_{c}")
                        nc.vector.tensor_scalar(out=ch3[:], in0=C_c, scalar1=1.0,
                                                scalar2=None, op0=Alu.max)
                        nc.vector.reciprocal(out=ch3[:], in_=ch3[:])
                        nc.vector.tensor_tensor(out=ch3[:], in0=I_c, in1=ch3[:], op=Alu.mult)
                        nc.vector.tensor_tensor(out=ch3[:], in0=ch3[:], in1=mask[:], op=Alu.mult)
                        nc.sync.dma_start(out=bev_dram[3, 125 * c:125 * (c + 1), :], in_=ch3[:])

            # ---------------- CNN (replicated, fp32) -------------------------
            with tc.tile_pool(name="cnnw", bufs=1) as wp:
                W1s = wp.tile([4, 9 * 32], f32, tag="w1")
                W2s = wp.tile([32, 9 * 64], f32, tag="w2")
                W3s = wp.tile([64, 9 * 64], f32, tag="w3")
                nc.sync.dma_start(out=W1s[:], in_=w1_t[:])
                nc.sync.dma_start(out=W2s[:], in_=w2_t[:])
                nc.sync.dma_start(out=W3s[:], in_=w3_t[:])
                gbs = []
                for i, (cn, gt) in enumerate(((32, gb1_t), (64, gb2_t), (64, gb3_t))):
                    g = wp.tile([cn, 2], f32, tag=f"gb{i}", name=f"gb{i}")
                    nc.sync.dma_start(out=g[:], in_=gt[:])
                    gbs.append(g)

                img1 = wp.tile([4, IMG_PAD], f32, tag="img1")
                img2 = wp.tile([32, IMG_PAD], f32, tag="img2")
                img3 = wp.tile([64, IMG_PAD], bf, tag="img3")
                nc.vector.memset(img1[:], 0.0)
                nc.vector.memset(img2[:], 0.0)
                nc.vector.memset(img3[:], 0.0)
                # interior of img1 <- bev
                img1_in = img1[:, 253:253 + 250 * 252].rearrange(
                    "p (r w) -> p r w", w=252)[:, :, 0:250]
                nc.sync.dma_start(out=img1_in, in_=bev_dram[:])

                def conv_layer(img_src, Ws, gbt, cin, cout, img_dst, src_bf):
                    from concourse.bass import ds as _ds
                    with tc.tile_pool(name="cl", bufs=4) as lp, \
                         tc.tile_pool(name="clp", bufs=2, space="PSUM") as lps:
                        s1p = lp.tile([cout, 128], f32, tag="s1p")
                        s2p = lp.tile([cout, 128], f32, tag="s2p")
                        # pass 1: conv + stats only
                        with tc.For_i(0, N_CHUNK, U_CNN) as j0:
                            for jj in range(U_CNN):
                                j = j0 + jj
                                base = j * 504
                                if src_bf:
                                    win = lp.tile([cin, 1012], f32, tag="win")
                                    nc.vector.tensor_copy(out=win[:, 0:1010],
                                                          in_=img_src[:, _ds(base, 1010)])
                                psc = lps.tile([cout, 504], f32, tag="psc")
                                for tap in range(9):
                                    off = (tap // 3) * 252 + tap % 3
                                    rhs = (win[:, off:off + 504] if src_bf
                                           else img_src[:, _ds(base + off, 504)])
                                    nc.tensor.matmul(out=psc[:],
                                                     lhsT=Ws[:, cout * tap:cout * (tap + 1)],
                                                     rhs=rhs, start=(tap == 0), stop=(tap == 8))
                                pv = psc[:].rearrange("p (r w) -> p r w", w=252)[:, :, 0:250]
                                scr = lp.tile([cout, 504], f32, tag="scr")
                                sv = scr[:].rearrange("p (r w) -> p r w", w=252)[:, :, 0:250]
                                nc.vector.tensor_scalar(out=sv, in0=pv, scalar1=1.0,
                                                        scalar2=None, op0=Alu.mult,
                                                        accum_out=s1p[:, _ds(j, 1)])
                                nc.vector.tensor_tensor_reduce(out=sv, in0=pv, in1=pv,
                                                               scale=1.0, scalar=0.0,
                                                               op0=Alu.mult, op1=Alu.add,
                                                               accum_out=s2p[:, _ds(j, 1)])
                        # stats -> scale/shift
                        S1 = lp.tile([cout, 1], f32, tag="S1")
                        S2 = lp.tile([cout, 1], f32, tag="S2")
                        nc.vector.tensor_reduce(out=S1[:], in_=s1p[:, 0:N_CHUNK], axis=mybir.AxisListType.X, op=Alu.add)
                        nc.vector.tensor_reduce(out=S2[:], in_=s2p[:, 0:N_CHUNK], axis=mybir.AxisListType.X, op=Alu.add)
                        mu = lp.tile([cout, 1], f32, tag="mu")
                        var = lp.tile([cout, 1], f32, tag="var")
                        nc.vector.tensor_scalar(out=mu[:], in0=S1[:], scalar1=1.0 / 62500.0,
                                                scalar2=None, op0=Alu.mult)
                        nc.vector.tensor_scalar(out=var[:], in0=S2[:], scalar1=1.0 / 62500.0,
                                                scalar2=None, op0=Alu.mult)
                        musq = lp.tile([cout, 1], f32, tag="musq")
                        nc.vector.tensor_tensor(out=musq[:], in0=mu[:], in1=mu[:], op=Alu.mult)
                        nc.vector.tensor_tensor(out=var[:], in0=var[:], in1=musq[:], op=Alu.subtract)
                        sd = lp.tile([cout, 1], f32, tag="sd")
                        nc.scalar.activation(out=sd[:], in_=var[:], func=Act.Sqrt, bias=epsc[0:cout, :])
                        scl = lp.tile([cout, 1], f32, tag="scl")
                        sh = lp.tile([cout, 1], f32, tag="sh")
                        nc.vector.tensor_tensor(out=scl[:], in0=gbt[:, 0:1], in1=sd[:], op=Alu.divide)
                        nc.vector.tensor_tensor(out=sh[:], in0=mu[:], in1=scl[:], op=Alu.mult)
                        nc.vector.tensor_tensor(out=sh[:], in0=gbt[:, 1:2], in1=sh[:], op=Alu.subtract)
                        # pass 2: conv + BN + relu -> img_dst (or y3_dram)
                        with tc.For_i(0, N_CHUNK, U_CNN) as j0:
                            for jj in range(U_CNN):
                                j = j0 + jj
                                base = j * 504
                                if src_bf:
                                    win = lp.tile([cin, 1012], f32, tag="win2")
                                    nc.vector.tensor_copy(out=win[:, 0:1010],
                                                          in_=img_src[:, _ds(base, 1010)])
                                psc = lps.tile([cout, 504], f32, tag="psc2")
                                for tap in range(9):
                                    off = (tap // 3) * 252 + tap % 3
                                    rhs = (win[:, off:off + 504] if src_bf
                                           else img_src[:, _ds(base + off, 504)])
                                    nc.tensor.matmul(out=psc[:],
                                                     lhsT=Ws[:, cout * tap:cout * (tap + 1)],
                                                     rhs=rhs, start=(tap == 0), stop=(tap == 8))
                                pv = psc[:].rearrange("p (r w) -> p r w", w=252)[:, :, 0:250]
                                if img_dst is not None:
                                    dst = img_dst[:, _ds(base + 253, 504)].rearrange(
                                        "p (r w) -> p r w", w=252)[:, :, 0:250]
                                    nc.scalar.activation(out=dst, in_=pv, func=Act.Relu,
                                                         scale=scl[:], bias=sh[:])
                                else:
                                    stage = lp.tile([cout, 504], f32, tag="stage")
                                    stv = stage[:].rearrange("p (r w) -> p r w",
                                                             w=252)[:, :, 0:250]
                                    nc.scalar.activation(out=stv, in_=pv, func=Act.Relu,
                                                         scale=scl[:], bias=sh[:])
                                    nc.sync.dma_start(out=y3_dram[:, 2 * j:2 * j + 2, :],
                                                      in_=stage[:])

                conv_layer(img1, W1s, gbs[0], 4, 32, img2, False)
                conv_layer(img2, W2s, gbs[1], 32, 64, img3, False)
                conv_layer(img3, W3s, gbs[2], 64, 64, None, True)

            # final: write this core's 32-row slice
            from concourse.bass import ds as _ds
            tmp_r = nc.sync.alloc_register("roff_reg")
            nc.sync.reg_load(tmp_r, roff_t[0:1, 0:1])
            roff = nc.sync.snap(tmp_r, donate=True, min_val=0, max_val=218)
            nc.sync.dma_start(out=out_t[:], in_=y3_dram[:, _ds(roff, 32), 0:250])

    nc.compile()
    return nc


def _get_program():
    if "nc" not in _CACHE:
        _CACHE["nc"] = _build()
    return _CACHE["nc"]


def _host_prep(inputs):
    pts = np.ascontiguousarray(np.asarray(inputs["points"], dtype=np.float32))
    n = pts.shape[0]
    pad = np.empty((N_PAD, 4), np.float32)
    pad[:n] = pts
    if N_PAD > n:
        pad[n:] = np.array([1e9, 1e9, 0.0, 0.0], np.float32)
    percore = pad.reshape(N_CORES, 128, T_TILES * 4)

    import ml_dtypes
    iota = np.broadcast_to(np.arange(250, dtype=np.float32), (128, 250))
    iota = np.ascontiguousarray(iota.astype(ml_dtypes.bfloat16))
    ident = np.eye(128, dtype=np.float32)
    i125 = np.stack([np.arange(125, dtype=np.float32),
                     np.arange(125, 250, dtype=np.float32)], axis=1)
    i125 = np.ascontiguousarray(i125)

    def wprep(w, cin, cout):
        # [cout, cin, 3, 3] -> [cin, 9*cout] tap-major
        return np.ascontiguousarray(
            np.asarray(w, np.float32).transpose(1, 2, 3, 0).reshape(cin, 9 * cout))

    def gbprep(g, b):
        return np.ascontiguousarray(
            np.stack([np.asarray(g, np.float32), np.asarray(b, np.float32)], axis=1))

    common = {
        "iota250": iota, "ident": ident, "i125": i125,
        "w1": wprep(inputs["w1"], 4, 32),
        "w2": wprep(inputs["w2"], 32, 64),
        "w3": wprep(inputs["w3"], 64, 64),
        "gb1": gbprep(inputs["g1"], inputs["beta1"]),
        "gb2": gbprep(inputs["g2"], inputs["beta2"]),
        "gb3": gbprep(inputs["g3"], inputs["beta3"]),
    }
    in_maps = []
    for k in range(N_CORES):
        m = dict(common)
        m["pts"] = percore[k]
        m["roff"] = np.array([[min(32 * k, 218)]], np.uint32)
        in_maps.append(m)
    return in_maps


def kernel(**inputs) -> np.ndarray:
    global LAST_HW_EXEC_NS
    from concourse import bass_utils
    nc = _get_program()
    in_maps = _host_prep(inputs)
    trace = os.environ.get("BEV_TRACE", "1") == "1"
    res = bass_utils.run_bass_kernel_spmd(
        nc, in_maps, core_ids=list(range(N_CORES)), trace=trace)
    _CACHE["last_res"] = res
    if getattr(res, "exec_time_ns", None):
        LAST_HW_EXEC_NS = int(res.exec_time_ns)
    out = np.empty((1, 64, 250, 250), np.float32)
    for k in range(N_CORES):
        o = res.results[k]["out"]
        if k < 7:
            out[0, :, 32 * k:32 * k + 32] = o
        else:
            out[0, :, 224:250] = o[:, 6:32]
    return out
